# revision 15
# baseline (speedup 1.0000x reference)
"""Trainium2 Bass kernel for nn_MultiHeadedAttention (B=4, S=1024, D=1024, H=16).

Sharding: 8 cores = 4 batches x 2 head-halves (8 heads each). The reference's
row-major reshape after [B,H,S,d] means output row r = h*64 + s//16 depends
only on head h, so head sharding needs no collective: each core computes a
[512, 1024] row-block of its batch's output.

All matmul operands are bf16 (host-converted): same PE rate as fp32r
(1 cycle/row at N>=256) but half the HBM traffic (~12MB/core), and no
in-flight dtype cast -- so every bulk load rides the two hardware DGE
rings (SP + Activation) instead of the slow gpsimd SWDGE ring. Pools are
sized so all input DMAs are in flight from t=0 with no slot recycling.

PSUM layout: psA (2 x [128,1024]) hosts the projection chunks, the score
tiles, and the final-projection accumulators; psB (4 x [65,512]) hosts
only the four PV accumulators of the current pair, so the pv ring never
couples to proj/fp allocations. Matmuls are ordered so consecutive
instructions share the stationary operand (one ldweights per 2 matmuls
in proj/scores/final).

Per-core pipeline (all matmuls contract on the partition dim):
  QT/KT = WxT.T @ XxT          -> [j, s] layout (head dims on partitions)
  V     = XvT.T @ WvT          -> [s, j] natural layout, augmented with a
                                  ones column per head (row 64 of PV psum
                                  then accumulates the softmax denominator)
  scoresT[k, q] = KT_h.T @ QT_h  (q in s16-major order so PV output lands in
                                  the layout the final reshape needs)
  wT = exp(0.125 * scoresT)      (mask is a no-op unless mask@mask.T has
                                  zeros; host checks and enables a penalty-mult
                                  fallback path in that case)
  xT'[dd|sum, q] = V_aug.T @ wT  (accumulated over k tiles)
  lhsT = xT'[0:64] * (1/sum)     (DVE copy into x_block.T layout, 2 heads
                                  side by side)
  out  = lhsT.T @ WoT            -> [128 rows, 1024] per head pair, DMA'd out.
"""

import contextlib

import numpy as np
import ml_dtypes

import concourse.bass as bass
import concourse.bacc as bacc
import concourse.tile as tile
from concourse import mybir
from concourse.bass_utils import run_bass_kernel_spmd

F32 = mybir.dt.float32
BF16 = mybir.dt.bfloat16
NP_BF16 = ml_dtypes.bfloat16



B, S, D, H = 4, 1024, 1024, 16
d_head = D // H  # 64
HPC = 8          # heads per core
JC = HPC * d_head  # 512 columns of W per core

_cached = {}


def build_program(use_mask: bool, loop_n=None, loads_in_loop=True,
                  phase='full'):
    nc = bacc.Bacc(None, target_bir_lowering=False, debug=False)

    xqT = nc.dram_tensor("xqT", [D, S], BF16, kind="ExternalInput").ap()
    xkT = nc.dram_tensor("xkT", [D, S], BF16, kind="ExternalInput").ap()
    xvT = nc.dram_tensor("xvT", [D, S], BF16, kind="ExternalInput").ap()
    wqT = nc.dram_tensor("wqT", [D, JC], BF16, kind="ExternalInput").ap()
    wkT = nc.dram_tensor("wkT", [D, JC], BF16, kind="ExternalInput").ap()
    wvT = nc.dram_tensor("wvT", [D, JC], BF16, kind="ExternalInput").ap()
    bq_col = nc.dram_tensor("bq_col", [128, 4], F32, kind="ExternalInput").ap()
    bk_col = nc.dram_tensor("bk_col", [128, 4], F32, kind="ExternalInput").ap()
    bv_bc = nc.dram_tensor("bv_bc", [128, JC], F32, kind="ExternalInput").ap()
    woT = nc.dram_tensor("woT", [D, D], BF16, kind="ExternalInput").ap()
    if use_mask:
        pen = nc.dram_tensor("pen", [S, S], BF16, kind="ExternalInput").ap()
    out = nc.dram_tensor("out", [JC, D], F32, kind="ExternalOutput").ap()

    with tile.TileContext(nc) as tc:
        with (
            tc.tile_pool(name="big", bufs=8) as big,     # x + wo tiles, 1MB bf16
            tc.tile_pool(name="wp", bufs=6) as wp,       # w half-tiles, 0.5MB
            tc.tile_pool(name="penp", bufs=2) as penp,
            tc.tile_pool(name="qt", bufs=4) as qt_p,
            tc.tile_pool(name="kt", bufs=4) as kt_p,
            tc.tile_pool(name="va", bufs=8) as va_p,
            tc.tile_pool(name="wT", bufs=6) as wT_p,
            tc.tile_pool(name="lh", bufs=2) as lh_p,
            tc.tile_pool(name="outp", bufs=2) as outp,
            tc.tile_pool(name="small", bufs=6) as smallp,
            tc.tile_pool(name="psA", bufs=2, space="PSUM") as psA,
            tc.tile_pool(name="psB", bufs=4, space="PSUM") as psB,
        ):
            def load_wide(dram, pool, tag, ncols, rows_per_tile, n, eng):
                # [128, rows_per_tile*ncols] tiles: row-blocks (contraction
                # tiles dt) interleaved along free so each DRAM matrix
                # needs few big DMAs
                a = rows_per_tile
                ts = []
                for i in range(n):
                    t = pool.tile([128, a * ncols], BF16, tag=tag, name=tag)
                    src_ap = (dram[i * a * 128:(i + 1) * a * 128, :]
                              .rearrange("(a p) s -> p a s", a=a))
                    eng.dma_start(t[:], src_ap)
                    ts.append(t)
                return lambda dt: ts[dt // a][:, (dt % a) * ncols:
                                              (dt % a + 1) * ncols]

            def emit_loads():
                ld = {}
                # biases lead the scalar ring (tiny, needed first)
                ld["bq"] = smallp.tile([128, 4], F32, tag="bias", bufs=2, name="bq_sb")
                nc.scalar.dma_start(ld["bq"][:], bq_col[:])
                ld["bk"] = smallp.tile([128, 4], F32, tag="bias", bufs=2, name="bk_sb")
                nc.scalar.dma_start(ld["bk"][:], bk_col[:])
                ld["bv"] = smallp.tile([128, JC], F32, tag="biasr", bufs=1, name="bv_sb")
                nc.scalar.dma_start(ld["bv"][:], bv_bc[:])
                # bulk streams split across the two HWDGE rings, in
                # consumption order per ring
                ld["wq"] = load_wide(wqT, wp, "w", JC, 4, 2, nc.scalar)
                ld["xq"] = load_wide(xqT, big, "x", S, 4, 2, nc.sync)
                ld["wk"] = load_wide(wkT, wp, "w", JC, 4, 2, nc.scalar)
                ld["xk"] = load_wide(xkT, big, "x", S, 4, 2, nc.sync)
                ld["wv"] = load_wide(wvT, wp, "w", JC, 4, 2, nc.sync)
                ld["xv"] = load_wide(xvT, big, "x", S, 4, 2, nc.scalar)
                ld["wo"] = load_wide(woT, big, "x", D, 4, 2, nc.scalar)
                if use_mask:
                    ld["pen"] = load_wide(pen, penp, "pen", S, 4, 2, nc.gpsimd)
                return ld

            if not loads_in_loop:
                LD = emit_loads()

            _loop = tc.For_i(0, loop_n) if loop_n else contextlib.nullcontext()
            with _loop:
                if loads_in_loop:
                    LD = emit_loads()
                wt_q, xt_q = LD["wq"], LD["xq"]
                wt_k, xt_k = LD["wk"], LD["xk"]
                wvt, xvt = LD["wv"], LD["xv"]
                wo_t = LD["wo"]
                bq_sb, bk_sb, bv_sb = LD["bq"], LD["bk"], LD["bv"]
                pen_t = LD.get("pen")

                warm = smallp.tile([1, 8], F32, tag="warm", bufs=1)
                nc.vector.memset(warm[:], 0.0)
                nc.scalar.activation(warm[:], warm[:],
                                     mybir.ActivationFunctionType.Exp)

                # V_aug tiles: stable slots; ones columns seeded early (the
                # per-iteration bias-add only rewrites the 64 value columns)
                VAS = 128  # head stride: cols 64-127 stay 1.0 -> 64 denominator copies
                VA = [va_p.tile([128, 8 * VAS], BF16, name="va") for _ in range(8)]
                for va in VA:
                    nc.vector.memset(va[:], 1.0)

                def proj_jt(wt, xt, bias_sb, dst, jt):
                    # one [128,1024] psA tile per jt; st halves; dt outer so
                    # each weight tile is loaded once for both st matmuls
                    ps = psA.tile([128, 1024], F32, tag="sc", name="ps")
                    for dt in range(8):
                        w_ap = wt(dt)[:, jt * 128:(jt + 1) * 128]
                        for st in range(2):
                            nc.tensor.matmul(
                                ps[:, st * 512:(st + 1) * 512],
                                lhsT=w_ap,
                                rhs=xt(dt)[:, st * 512:(st + 1) * 512],
                                start=(dt == 0),
                                stop=(dt == 7),
                            )
                    for st in range(2):
                        nc.vector.tensor_scalar_add(
                            dst[jt][:, st * 512:(st + 1) * 512],
                            ps[:, st * 512:(st + 1) * 512],
                            bias_sb[:, jt:jt + 1],
                        )

                def proj_qk(wt, xt, bias_sb, dst_pool):
                    dst = [dst_pool.tile([128, S], BF16, tag="dst", name="dst") for _ in range(4)]
                    for jt in range(4):
                        proj_jt(wt, xt, bias_sb, dst, jt)
                    return dst

                QT = proj_qk(wt_q, xt_q, bq_sb, qt_p)
                KT = proj_qk(wt_k, xt_k, bk_sb, kt_p)

                # ---- V projection -> V_aug [s, 8*65] (65th col per head = 1.0)
                for sp in range(4):  # st pairs; one psA tile per pair
                    ps = psA.tile([128, 1024], F32, tag="sc", name="vps")
                    for half in range(2):
                        st = 2 * sp + half
                        for dt in range(8):
                            nc.tensor.matmul(
                                ps[:, half * 512:(half + 1) * 512],
                                lhsT=xvt(dt)[:, st * 128:(st + 1) * 128],
                                rhs=wvt(dt),
                                start=(dt == 0),
                                stop=(dt == 7),
                            )
                    for half in range(2):
                        st = 2 * sp + half
                        nc.vector.tensor_tensor(
                            VA[st][:].rearrange("p (h c) -> p h c", h=8)[:, :, 0:64],
                            ps[:, half * 512:(half + 1) * 512]
                                .rearrange("p (h c) -> p h c", h=8),
                            bv_sb[:].rearrange("p (h c) -> p h c", h=8),
                            op=mybir.AluOpType.add,
                        )


                def QT_perm(hl, qch):
                    # rhs [64, 512] with q in s16-major order:
                    # col j reads s = q16*16 + s16, s16 = qch*8 + j//64, q16 = j%64
                    tile_ = QT[hl // 2]
                    po = (hl % 2) * 64
                    ap = tile_[po:po + 64, :].rearrange("p (q s) -> p s q", s=16)
                    return ap[:, qch * 8:(qch + 1) * 8, :]

                def KT_ap(hl, kt):
                    tile_ = KT[hl // 2]
                    po = (hl % 2) * 64
                    return tile_[po:po + 64, kt * 128:(kt + 1) * 128]

                PV_LAG = 2  # kt-steps the PV matmuls trail scores/exp

                def attention(p, hook_norm=None, hook_fp=None, lag=None):
                    lag = PV_LAG if lag is None else lag
                    hA, hB = 2 * p, 2 * p + 1
                    pv = {}
                    wstash = {}
                    for step in range(8 + lag):
                        if step == 1 and hook_norm is not None:
                            hook_norm()
                        if step == 4 and hook_fp is not None:
                            hook_fp()
                        if step < 8:
                            kt = step
                            scA = psA.tile([128, 1024], F32, tag="sc")
                            scB = psA.tile([128, 1024], F32, tag="sc")
                            # per head: both qch back-to-back (shared lhsT,
                            # and exp can start after the 2nd matmul)
                            for hl, sc in ((hA, scA), (hB, scB)):
                                for qch in range(2):
                                    nc.tensor.matmul(
                                        sc[:, qch * 512:(qch + 1) * 512],
                                        lhsT=KT_ap(hl, kt),
                                        rhs=QT_perm(hl, qch),
                                        start=True, stop=True,
                                    )
                            wA = wT_p.tile([128, 1024], BF16, tag="wT")
                            wB = wT_p.tile([128, 1024], BF16, tag="wT")
                            nc.scalar.activation(wA[:], scA[:],
                                                 mybir.ActivationFunctionType.Exp,
                                                 scale=0.125)
                            nc.scalar.activation(wB[:], scB[:],
                                                 mybir.ActivationFunctionType.Exp,
                                                 scale=0.125)
                            if use_mask:
                                # multiply by the 0/1 keep-mask (pen[k, q]) with
                                # the same s16-major q permutation as wT columns
                                pap = pen_t(kt).rearrange("p (q s) -> p s q", s=16)
                                for w_ in (wA, wB):
                                    nc.vector.tensor_tensor(
                                        w_[:].rearrange("p (s q) -> p s q", s=16),
                                        w_[:].rearrange("p (s q) -> p s q", s=16),
                                        pap, op=mybir.AluOpType.mult,
                                    )
                            wstash[kt] = (wA, wB)
                        if phase in ('proj', 'scexp'):
                            wstash.clear()
                            continue
                        if step >= lag:
                            kt = step - lag
                            wA, wB = wstash.pop(kt)
                            # per head: both qch consecutive (shared VA lhsT);
                            # pv[i] key: i = 2*hloc + qch
                            for i, (hl, wt_, qch) in enumerate(
                                [(hA, wA, 0), (hA, wA, 1), (hB, wB, 0), (hB, wB, 1)]
                            ):
                                if kt == 0:
                                    pv[i] = psB.tile([128, 512], F32, tag="ps1", name="pv")
                                nc.tensor.matmul(
                                    pv[i][:],
                                    lhsT=VA[kt][:, hl * VAS:(hl + 1) * VAS],
                                    rhs=wt_[:, qch * 512:(qch + 1) * 512],
                                    start=(kt == 0), stop=(kt == 7),
                                )
                    return pv

                def tail_norm(p, pv):
                    hA, hB = 2 * p, 2 * p + 1
                    # pv[i]: i = 2*hloc + qch
                    # normalize + shuffle into final-projection lhsT layout
                    lh = lh_p.tile([128, 1024], BF16)
                    for hloc, hl in enumerate((hA, hB)):
                        rs = smallp.tile([64, 1024], F32, tag="rs", bufs=2,
                                         name="rs")
                        for qch in range(2):
                            i = 2 * hloc + qch
                            nc.vector.reciprocal(
                                rs[:, qch * 512:(qch + 1) * 512],
                                pv[i][64:128, :])
                        rcv = rs[:].rearrange("p (s q) -> p s q", s=16)
                        for qch in range(2):
                            i = 2 * hloc + qch
                            src = pv[i][0:64, :].rearrange("p (s q) -> p s q", s=8)
                            for par, off in ((0, 0), (1, 64)):  # even/odd s16
                                # lh layout: [part, (ct 8)(head 2)(q16 64)] so the
                                # final matmul's lhsT tile ct is one contiguous
                                # 128-col block (walrus: stationary AP needs a
                                # single free dim)
                                dst = lh[off:off + 64, :].rearrange(
                                    "p (c m) -> p c m", c=8
                                )[:, qch * 4:(qch + 1) * 4,
                                  hloc * 64:(hloc + 1) * 64]
                                nc.vector.tensor_tensor(
                                    dst,
                                    src[:, par::2, :],
                                    rcv[:, qch * 8 + par:qch * 8 + 8:2, :],
                                    op=mybir.AluOpType.mult,
                                )

                    return lh

                def tail_fp(p, lh):
                    # final projection: out rows p*128 .. p*128+128
                    # one [128,1024] psA tile; ot halves; ct outer so each
                    # lh slice is loaded once for both ot matmuls
                    ob = outp.tile([128, 1024], F32)
                    fp = psA.tile([128, 1024], F32, tag="sc", name="fp")
                    for ct in range(8):
                        lh_ap = lh[:, ct * 128:(ct + 1) * 128]
                        for ot in range(2):
                            nc.tensor.matmul(
                                fp[:, ot * 512:(ot + 1) * 512],
                                lhsT=lh_ap,
                                rhs=wo_t(ct)[:, ot * 512:(ot + 1) * 512],
                                start=(ct == 0), stop=(ct == 7),
                            )
                    for ot in range(2):
                        nc.vector.tensor_copy(
                            ob[:, ot * 512:(ot + 1) * 512],
                            fp[:, ot * 512:(ot + 1) * 512])
                    nc.sync.dma_start(out[p * 128:(p + 1) * 128, :], ob[:])

                if phase == 'proj':
                    dbg = outp.tile([128, 1024], F32, name="dbg")
                    nc.vector.tensor_copy(dbg[:, 0:260], VA[0][:].bitcast(F32)[:, 0:260])
                    nc.vector.tensor_copy(dbg[:, 0:512], QT[0][:].bitcast(F32)[:, 0:512])
                    nc.vector.tensor_copy(dbg[:, 0:512], KT[0][:].bitcast(F32)[:, 0:512])
                    nc.sync.dma_start(out[0:128, :], dbg[:])
                elif phase == 'scexp':
                    for p in range(4):
                        attention(p)
                    dbg = outp.tile([128, 1024], F32, name="dbg")
                    nc.vector.tensor_copy(dbg[:, 0:260], VA[0][:].bitcast(F32)[:, 0:260])
                    nc.sync.dma_start(out[0:128, :], dbg[:])
                elif phase == 'pvonly':
                    # PV included; drain pv accumulators with cheap DVE copies
                    dbg = outp.tile([128, 1024], F32, name="dbg")
                    for p in range(4):
                        pv = attention(p)
                        for i in range(2):
                            nc.vector.tensor_copy(dbg[:, :], pv[i][:])
                    nc.sync.dma_start(out[0:128, :], dbg[:])
                else:
                    # software-pipeline: emit pair p's attention before pair
                    # p-1's norm/final so PE gap-fills the ACT-paced exp phase
                    pending = None
                    for p in range(4):
                        hn = hf = None
                        if pending is not None:
                            pp, ppv = pending
                            box = {}

                            def hn(pp=pp, ppv=ppv, box=box):
                                box["lh"] = tail_norm(pp, ppv)

                            def hf(pp=pp, box=box):
                                tail_fp(pp, box["lh"])

                        pv = attention(p, hn, hf)
                        pending = (p, pv)
                    pp, ppv = pending
                    tail_fp(pp, tail_norm(pp, ppv))

    nc.compile()
    return nc


def make_in_maps(query, key, value, mask, Wq, bq, Wk, bk, Wv, bv, Wo,
                 pen_b=None):
    woT = np.ascontiguousarray(Wo.T).astype(NP_BF16)
    maps = []
    for c in range(8):
        b, hf = c // 2, c % 2
        sl = slice(hf * JC, (hf + 1) * JC)
        m = {
            "xqT": np.ascontiguousarray(query[b].T).astype(NP_BF16),
            "xkT": np.ascontiguousarray(key[b].T).astype(NP_BF16),
            "xvT": np.ascontiguousarray(value[b].T).astype(NP_BF16),
            "wqT": np.ascontiguousarray(Wq[sl].T).astype(NP_BF16),
            "wkT": np.ascontiguousarray(Wk[sl].T).astype(NP_BF16),
            "wvT": np.ascontiguousarray(Wv[sl].T).astype(NP_BF16),
            "bq_col": np.ascontiguousarray(bq[sl].reshape(4, 128).T),
            "bk_col": np.ascontiguousarray(bk[sl].reshape(4, 128).T),
            "bv_bc": np.ascontiguousarray(
                np.broadcast_to(bv[sl].reshape(1, JC), (128, JC))),
            "woT": woT,
        }
        if pen_b is not None:
            m["pen"] = pen_b[b].astype(NP_BF16)
        maps.append(m)
    return maps


def kernel(query, key, value, mask, Wq, bq, Wk, bk, Wv, bv, Wo):
    query = np.asarray(query, np.float32)
    key = np.asarray(key, np.float32)
    value = np.asarray(value, np.float32)
    mask = np.asarray(mask, np.float32)

    m2d = mask[0]  # [B, S, 64]
    mm = np.stack([m2d[b] @ m2d[b].T for b in range(B)])  # [B, S, S]
    use_mask = bool((mm == 0).any())
    pen_b = None
    if use_mask:
        pen_b = np.where(mm == 0, np.float32(0.0), np.float32(1.0))
        pen_b = np.ascontiguousarray(pen_b, np.float32)

    if use_mask not in _cached:
        _cached[use_mask] = build_program(use_mask)
    nc = _cached[use_mask]

    in_maps = make_in_maps(query, key, value, mask,
                           np.asarray(Wq, np.float32), np.asarray(bq, np.float32),
                           np.asarray(Wk, np.float32), np.asarray(bk, np.float32),
                           np.asarray(Wv, np.float32), np.asarray(bv, np.float32),
                           np.asarray(Wo, np.float32), pen_b)
    res = run_bass_kernel_spmd(nc, in_maps, list(range(8)))

    out = np.empty((B, S, D), np.float32)
    for c in range(8):
        b, hf = c // 2, c % 2
        out[b, hf * JC:(hf + 1) * JC, :] = res.results[c]["out"]
    return out


# revision 16
# speedup vs baseline: 1.0559x; 1.0559x over previous
"""Trainium2 Bass kernel for nn_MultiHeadedAttention (B=4, S=1024, D=1024, H=16).

Sharding: 8 cores = 4 batches x 2 head-halves (8 heads each). The reference's
row-major reshape after [B,H,S,d] means output row r = h*64 + s//16 depends
only on head h, so head sharding needs no collective: each core computes a
[512, 1024] row-block of its batch's output.

All matmul operands are bf16 (host-converted): same PE rate as fp32r
(1 cycle/row at N>=256) but half the HBM traffic (~12MB/core), and no
in-flight dtype cast -- so every bulk load rides the two hardware DGE
rings (SP + Activation) instead of the slow gpsimd SWDGE ring. Pools are
sized so all input DMAs are in flight from t=0 with no slot recycling.

PSUM layout: psA (2 x [128,1024]) hosts the projection chunks, the score
tiles, and the final-projection accumulators; psB (4 x [65,512]) hosts
only the four PV accumulators of the current pair, so the pv ring never
couples to proj/fp allocations. Matmuls are ordered so consecutive
instructions share the stationary operand (one ldweights per 2 matmuls
in proj/scores/final).

Per-core pipeline (all matmuls contract on the partition dim):
  QT/KT = WxT.T @ XxT          -> [j, s] layout (head dims on partitions)
  V     = XvT.T @ WvT          -> [s, j] natural layout, augmented with a
                                  ones column per head (row 64 of PV psum
                                  then accumulates the softmax denominator)
  scoresT[k, q] = KT_h.T @ QT_h  (q in s16-major order so PV output lands in
                                  the layout the final reshape needs)
  wT = exp(0.125 * scoresT)      (mask is a no-op unless mask@mask.T has
                                  zeros; host checks and enables a penalty-mult
                                  fallback path in that case)
  xT'[dd|sum, q] = V_aug.T @ wT  (accumulated over k tiles)
  lhsT = xT'[0:64] * (1/sum)     (DVE copy into x_block.T layout, 2 heads
                                  side by side)
  out  = lhsT.T @ WoT            -> [128 rows, 1024] per head pair, DMA'd out.
"""

import contextlib

import numpy as np
import ml_dtypes

import concourse.bass as bass
import concourse.bacc as bacc
import concourse.tile as tile
from concourse import mybir
from concourse.bass_utils import run_bass_kernel_spmd

F32 = mybir.dt.float32
BF16 = mybir.dt.bfloat16
NP_BF16 = ml_dtypes.bfloat16



B, S, D, H = 4, 1024, 1024, 16
d_head = D // H  # 64
HPC = 8          # heads per core
JC = HPC * d_head  # 512 columns of W per core

_cached = {}


def build_program(use_mask: bool, loop_n=None, loads_in_loop=True,
                  phase='full'):
    nc = bacc.Bacc(None, target_bir_lowering=False, debug=False)

    xqT = nc.dram_tensor("xqT", [D, S], BF16, kind="ExternalInput").ap()
    xkT = nc.dram_tensor("xkT", [D, S], BF16, kind="ExternalInput").ap()
    xvT = nc.dram_tensor("xvT", [D, S], BF16, kind="ExternalInput").ap()
    wqT = nc.dram_tensor("wqT", [D, JC], BF16, kind="ExternalInput").ap()
    wkT = nc.dram_tensor("wkT", [D, JC], BF16, kind="ExternalInput").ap()
    wvT = nc.dram_tensor("wvT", [D, JC], BF16, kind="ExternalInput").ap()
    bq_col = nc.dram_tensor("bq_col", [128, 4], F32, kind="ExternalInput").ap()
    bk_col = nc.dram_tensor("bk_col", [128, 4], F32, kind="ExternalInput").ap()
    bv_bc = nc.dram_tensor("bv_bc", [128, JC], F32, kind="ExternalInput").ap()
    woT = nc.dram_tensor("woT", [D, D], BF16, kind="ExternalInput").ap()
    if use_mask:
        pen = nc.dram_tensor("pen", [S, S], BF16, kind="ExternalInput").ap()
    out = nc.dram_tensor("out", [JC, D], F32, kind="ExternalOutput").ap()

    with tile.TileContext(nc) as tc:
        with (
            tc.tile_pool(name="big", bufs=8) as big,     # x + wo tiles, 1MB bf16
            tc.tile_pool(name="wp", bufs=6) as wp,       # w half-tiles, 0.5MB
            tc.tile_pool(name="penp", bufs=2) as penp,
            tc.tile_pool(name="qt", bufs=4) as qt_p,
            tc.tile_pool(name="kt", bufs=4) as kt_p,
            tc.tile_pool(name="va", bufs=8) as va_p,
            tc.tile_pool(name="wT", bufs=12) as wT_p,
            tc.tile_pool(name="lh", bufs=2) as lh_p,
            tc.tile_pool(name="outp", bufs=2) as outp,
            tc.tile_pool(name="small", bufs=6) as smallp,
            tc.tile_pool(name="psA", bufs=4, space="PSUM") as psA,
            tc.tile_pool(name="psB", bufs=4, space="PSUM") as psB,
        ):
            def load_wide(dram, pool, tag, ncols, rows_per_tile, n, eng):
                # [128, rows_per_tile*ncols] tiles: row-blocks (contraction
                # tiles dt) interleaved along free so each DRAM matrix
                # needs few big DMAs
                a = rows_per_tile
                ts = []
                for i in range(n):
                    t = pool.tile([128, a * ncols], BF16, tag=tag, name=tag)
                    src_ap = (dram[i * a * 128:(i + 1) * a * 128, :]
                              .rearrange("(a p) s -> p a s", a=a))
                    eng.dma_start(t[:], src_ap)
                    ts.append(t)
                return lambda dt: ts[dt // a][:, (dt % a) * ncols:
                                              (dt % a + 1) * ncols]

            def emit_loads():
                ld = {}
                # biases lead the scalar ring (tiny, needed first)
                ld["bq"] = smallp.tile([128, 4], F32, tag="bias", bufs=2, name="bq_sb")
                nc.scalar.dma_start(ld["bq"][:], bq_col[:])
                ld["bk"] = smallp.tile([128, 4], F32, tag="bias", bufs=2, name="bk_sb")
                nc.scalar.dma_start(ld["bk"][:], bk_col[:])
                ld["bv"] = smallp.tile([128, JC], F32, tag="biasr", bufs=1, name="bv_sb")
                nc.scalar.dma_start(ld["bv"][:], bv_bc[:])
                # bulk streams split across the two HWDGE rings, in
                # consumption order per ring
                ld["wq"] = load_wide(wqT, wp, "w", JC, 4, 2, nc.scalar)
                ld["xq"] = load_wide(xqT, big, "x", S, 4, 2, nc.sync)
                ld["wk"] = load_wide(wkT, wp, "w", JC, 4, 2, nc.scalar)
                ld["xk"] = load_wide(xkT, big, "x", S, 4, 2, nc.sync)
                ld["wv"] = load_wide(wvT, wp, "w", JC, 4, 2, nc.sync)
                ld["xv"] = load_wide(xvT, big, "x", S, 4, 2, nc.scalar)
                ld["wo"] = load_wide(woT, big, "x", D, 4, 2, nc.scalar)
                if use_mask:
                    ld["pen"] = load_wide(pen, penp, "pen", S, 4, 2, nc.gpsimd)
                return ld

            if not loads_in_loop:
                LD = emit_loads()

            _loop = tc.For_i(0, loop_n) if loop_n else contextlib.nullcontext()
            with _loop:
                if loads_in_loop:
                    LD = emit_loads()
                wt_q, xt_q = LD["wq"], LD["xq"]
                wt_k, xt_k = LD["wk"], LD["xk"]
                wvt, xvt = LD["wv"], LD["xv"]
                wo_t = LD["wo"]
                bq_sb, bk_sb, bv_sb = LD["bq"], LD["bk"], LD["bv"]
                pen_t = LD.get("pen")

                warm = smallp.tile([1, 8], F32, tag="warm", bufs=1)
                nc.vector.memset(warm[:], 0.0)
                nc.scalar.activation(warm[:], warm[:],
                                     mybir.ActivationFunctionType.Exp)

                # V_aug tiles: stable slots; ones columns seeded early (the
                # per-iteration bias-add only rewrites the 64 value columns)
                VAS = 128  # head stride: cols 64-127 stay 1.0 -> 64 denominator copies
                VA = [va_p.tile([128, 8 * VAS], BF16, name="va") for _ in range(8)]
                for va in VA:
                    nc.vector.memset(va[:], 1.0)

                def proj_jt(wt, xt, bias_sb, dst, jt):
                    for st in range(2):
                        ps = psA.tile([128, 512], F32, tag="sc", name="ps")
                        for dt in range(8):
                            nc.tensor.matmul(
                                ps[:],
                                lhsT=wt(dt)[:, jt * 128:(jt + 1) * 128],
                                rhs=xt(dt)[:, st * 512:(st + 1) * 512],
                                start=(dt == 0),
                                stop=(dt == 7),
                            )
                        nc.vector.tensor_scalar_add(
                            dst[jt][:, st * 512:(st + 1) * 512], ps[:],
                            bias_sb[:, jt:jt + 1],
                        )

                def proj_qk(wt, xt, bias_sb, dst_pool):
                    dst = [dst_pool.tile([128, S], BF16, tag="dst", name="dst") for _ in range(4)]
                    for jt in range(4):
                        proj_jt(wt, xt, bias_sb, dst, jt)
                    return dst

                QT = proj_qk(wt_q, xt_q, bq_sb, qt_p)
                KT = proj_qk(wt_k, xt_k, bk_sb, kt_p)

                # ---- V projection -> V_aug (cols 64-127/head stay 1.0)
                for st in range(8):
                    ps = psA.tile([128, 512], F32, tag="sc", name="vps")
                    for dt in range(8):
                        nc.tensor.matmul(
                            ps[:],
                            lhsT=xvt(dt)[:, st * 128:(st + 1) * 128],
                            rhs=wvt(dt),
                            start=(dt == 0),
                            stop=(dt == 7),
                        )
                    nc.vector.tensor_tensor(
                        VA[st][:].rearrange("p (h c) -> p h c", h=8)[:, :, 0:64],
                        ps[:].rearrange("p (h c) -> p h c", h=8),
                        bv_sb[:].rearrange("p (h c) -> p h c", h=8),
                        op=mybir.AluOpType.add,
                    )


                def QT_perm(hl, qch):
                    # rhs [64, 512] with q in s16-major order:
                    # col j reads s = q16*16 + s16, s16 = qch*8 + j//64, q16 = j%64
                    tile_ = QT[hl // 2]
                    po = (hl % 2) * 64
                    ap = tile_[po:po + 64, :].rearrange("p (q s) -> p s q", s=16)
                    return ap[:, qch * 8:(qch + 1) * 8, :]

                def KT_ap(hl, kt):
                    tile_ = KT[hl // 2]
                    po = (hl % 2) * 64
                    return tile_[po:po + 64, kt * 128:(kt + 1) * 128]

                PV_LAG = 2  # kt-steps the PV matmuls trail scores/exp

                def attention(p, hook_norm=None, hook_fp=None, lag=None):
                    lag = PV_LAG if lag is None else lag
                    hA, hB = 2 * p, 2 * p + 1
                    pv = {}
                    wstash = {}
                    for step in range(8 + lag):
                        if step == 1 and hook_norm is not None:
                            hook_norm()
                        if step == 4 and hook_fp is not None:
                            hook_fp()
                        if step < 8:
                            kt = step
                            # per (head, qch): own [128,512] score tile + exp
                            # -> 4-deep psA ring, half-latency PE<->ACT links
                            ws = []
                            for hl in (hA, hB):
                                for qch in range(2):
                                    sc = psA.tile([128, 512], F32, tag="sc",
                                                  name="sc")
                                    nc.tensor.matmul(
                                        sc[:],
                                        lhsT=KT_ap(hl, kt),
                                        rhs=QT_perm(hl, qch),
                                        start=True, stop=True,
                                    )
                                    w_ = wT_p.tile([128, 512], BF16, tag="wT",
                                                   name="wT")
                                    nc.scalar.activation(
                                        w_[:], sc[:],
                                        mybir.ActivationFunctionType.Exp,
                                        scale=0.125)
                                    if use_mask:
                                        pap = (pen_t(kt)
                                               .rearrange("p (q s) -> p s q", s=16)
                                               [:, qch * 8:(qch + 1) * 8, :])
                                        nc.vector.tensor_tensor(
                                            w_[:].rearrange("p (s q) -> p s q", s=8),
                                            w_[:].rearrange("p (s q) -> p s q", s=8),
                                            pap, op=mybir.AluOpType.mult,
                                        )
                                    ws.append(w_)
                            wstash[kt] = ws
                        if phase in ('proj', 'scexp'):
                            wstash.clear()
                            continue
                        if step >= lag:
                            kt = step - lag
                            ws = wstash.pop(kt)
                            # ws order: (hA,q0),(hA,q1),(hB,q0),(hB,q1)
                            # pv[i] key: i = 2*hloc + qch
                            for i, (hl, w_) in enumerate(
                                [(hA, ws[0]), (hA, ws[1]),
                                 (hB, ws[2]), (hB, ws[3])]
                            ):
                                if kt == 0:
                                    pv[i] = psB.tile([128, 512], F32, tag="ps1", name="pv")
                                nc.tensor.matmul(
                                    pv[i][:],
                                    lhsT=VA[kt][:, hl * VAS:(hl + 1) * VAS],
                                    rhs=w_[:],
                                    start=(kt == 0), stop=(kt == 7),
                                )
                    return pv

                def tail_norm(p, pv):
                    hA, hB = 2 * p, 2 * p + 1
                    # pv[i]: i = 2*hloc + qch
                    # normalize + shuffle into final-projection lhsT layout
                    lh = lh_p.tile([128, 1024], BF16)
                    for hloc, hl in enumerate((hA, hB)):
                        rs = smallp.tile([64, 1024], F32, tag="rs", bufs=2,
                                         name="rs")
                        for qch in range(2):
                            i = 2 * hloc + qch
                            nc.vector.reciprocal(
                                rs[:, qch * 512:(qch + 1) * 512],
                                pv[i][64:128, :])
                        rcv = rs[:].rearrange("p (s q) -> p s q", s=16)
                        for qch in range(2):
                            i = 2 * hloc + qch
                            src = pv[i][0:64, :].rearrange("p (s q) -> p s q", s=8)
                            for par, off in ((0, 0), (1, 64)):  # even/odd s16
                                # lh layout: [part, (ct 8)(head 2)(q16 64)] so the
                                # final matmul's lhsT tile ct is one contiguous
                                # 128-col block (walrus: stationary AP needs a
                                # single free dim)
                                dst = lh[off:off + 64, :].rearrange(
                                    "p (c m) -> p c m", c=8
                                )[:, qch * 4:(qch + 1) * 4,
                                  hloc * 64:(hloc + 1) * 64]
                                nc.vector.tensor_tensor(
                                    dst,
                                    src[:, par::2, :],
                                    rcv[:, qch * 8 + par:qch * 8 + 8:2, :],
                                    op=mybir.AluOpType.mult,
                                )

                    return lh

                def tail_fp(p, lh):
                    # final projection: out rows p*128 .. p*128+128
                    # one [128,1024] psA tile; ot halves; ct outer so each
                    # lh slice is loaded once for both ot matmuls
                    ob = outp.tile([128, 1024], F32)
                    for ot in range(2):
                        fp = psA.tile([128, 512], F32, tag="sc", name="fp")
                        for ct in range(8):
                            nc.tensor.matmul(
                                fp[:],
                                lhsT=lh[:, ct * 128:(ct + 1) * 128],
                                rhs=wo_t(ct)[:, ot * 512:(ot + 1) * 512],
                                start=(ct == 0), stop=(ct == 7),
                            )
                        nc.vector.tensor_copy(
                            ob[:, ot * 512:(ot + 1) * 512], fp[:])
                    nc.sync.dma_start(out[p * 128:(p + 1) * 128, :], ob[:])

                if phase == 'proj':
                    dbg = outp.tile([128, 1024], F32, name="dbg")
                    nc.vector.tensor_copy(dbg[:, 0:260], VA[0][:].bitcast(F32)[:, 0:260])
                    nc.vector.tensor_copy(dbg[:, 0:512], QT[0][:].bitcast(F32)[:, 0:512])
                    nc.vector.tensor_copy(dbg[:, 0:512], KT[0][:].bitcast(F32)[:, 0:512])
                    nc.sync.dma_start(out[0:128, :], dbg[:])
                elif phase == 'scexp':
                    for p in range(4):
                        attention(p)
                    dbg = outp.tile([128, 1024], F32, name="dbg")
                    nc.vector.tensor_copy(dbg[:, 0:260], VA[0][:].bitcast(F32)[:, 0:260])
                    nc.sync.dma_start(out[0:128, :], dbg[:])
                elif phase == 'pvonly':
                    # PV included; drain pv accumulators with cheap DVE copies
                    dbg = outp.tile([128, 1024], F32, name="dbg")
                    for p in range(4):
                        pv = attention(p)
                        for i in range(4):
                            nc.vector.tensor_copy(dbg[0:65, 0:512], pv[i][:])
                    nc.sync.dma_start(out[0:128, :], dbg[:])
                else:
                    # software-pipeline: emit pair p's attention before pair
                    # p-1's norm/final so PE gap-fills the ACT-paced exp phase
                    pending = None
                    for p in range(4):
                        hn = hf = None
                        if pending is not None:
                            pp, ppv = pending
                            box = {}

                            def hn(pp=pp, ppv=ppv, box=box):
                                box["lh"] = tail_norm(pp, ppv)

                            def hf(pp=pp, box=box):
                                tail_fp(pp, box["lh"])

                        pv = attention(p, hn, hf)
                        pending = (p, pv)
                    pp, ppv = pending
                    tail_fp(pp, tail_norm(pp, ppv))

    nc.compile()
    return nc


def make_in_maps(query, key, value, mask, Wq, bq, Wk, bk, Wv, bv, Wo,
                 pen_b=None):
    woT = np.ascontiguousarray(Wo.T).astype(NP_BF16)
    maps = []
    for c in range(8):
        b, hf = c // 2, c % 2
        sl = slice(hf * JC, (hf + 1) * JC)
        m = {
            "xqT": np.ascontiguousarray(query[b].T).astype(NP_BF16),
            "xkT": np.ascontiguousarray(key[b].T).astype(NP_BF16),
            "xvT": np.ascontiguousarray(value[b].T).astype(NP_BF16),
            "wqT": np.ascontiguousarray(Wq[sl].T).astype(NP_BF16),
            "wkT": np.ascontiguousarray(Wk[sl].T).astype(NP_BF16),
            "wvT": np.ascontiguousarray(Wv[sl].T).astype(NP_BF16),
            "bq_col": np.ascontiguousarray(bq[sl].reshape(4, 128).T),
            "bk_col": np.ascontiguousarray(bk[sl].reshape(4, 128).T),
            "bv_bc": np.ascontiguousarray(
                np.broadcast_to(bv[sl].reshape(1, JC), (128, JC))),
            "woT": woT,
        }
        if pen_b is not None:
            m["pen"] = pen_b[b].astype(NP_BF16)
        maps.append(m)
    return maps


def kernel(query, key, value, mask, Wq, bq, Wk, bk, Wv, bv, Wo):
    query = np.asarray(query, np.float32)
    key = np.asarray(key, np.float32)
    value = np.asarray(value, np.float32)
    mask = np.asarray(mask, np.float32)

    m2d = mask[0]  # [B, S, 64]
    mm = np.stack([m2d[b] @ m2d[b].T for b in range(B)])  # [B, S, S]
    use_mask = bool((mm == 0).any())
    pen_b = None
    if use_mask:
        pen_b = np.where(mm == 0, np.float32(0.0), np.float32(1.0))
        pen_b = np.ascontiguousarray(pen_b, np.float32)

    if use_mask not in _cached:
        _cached[use_mask] = build_program(use_mask)
    nc = _cached[use_mask]

    in_maps = make_in_maps(query, key, value, mask,
                           np.asarray(Wq, np.float32), np.asarray(bq, np.float32),
                           np.asarray(Wk, np.float32), np.asarray(bk, np.float32),
                           np.asarray(Wv, np.float32), np.asarray(bv, np.float32),
                           np.asarray(Wo, np.float32), pen_b)
    res = run_bass_kernel_spmd(nc, in_maps, list(range(8)))

    out = np.empty((B, S, D), np.float32)
    for c in range(8):
        b, hf = c // 2, c % 2
        out[b, hf * JC:(hf + 1) * JC, :] = res.results[c]["out"]
    return out


# revision 19
# speedup vs baseline: 1.0604x; 1.0042x over previous
"""Trainium2 Bass kernel for nn_MultiHeadedAttention (B=4, S=1024, D=1024, H=16).

Sharding: 8 cores = 4 batches x 2 head-halves (8 heads each). The reference's
row-major reshape after [B,H,S,d] means output row r = h*64 + s//16 depends
only on head h, so head sharding needs no collective: each core computes a
[512, 1024] row-block of its batch's output.

All matmul operands are bf16 (host-converted): same PE rate as fp32r
(1 cycle/row at N>=256) but half the HBM traffic (~12MB/core), and no
in-flight dtype cast -- so every bulk load rides the two hardware DGE
rings (SP + Activation) instead of the slow gpsimd SWDGE ring. Pools are
sized so all input DMAs are in flight from t=0 with no slot recycling.

PSUM layout: psA (2 x [128,1024]) hosts the projection chunks, the score
tiles, and the final-projection accumulators; psB (4 x [65,512]) hosts
only the four PV accumulators of the current pair, so the pv ring never
couples to proj/fp allocations. Matmuls are ordered so consecutive
instructions share the stationary operand (one ldweights per 2 matmuls
in proj/scores/final).

Per-core pipeline (all matmuls contract on the partition dim):
  QT/KT = WxT.T @ XxT          -> [j, s] layout (head dims on partitions)
  V     = XvT.T @ WvT          -> [s, j] natural layout, augmented with a
                                  ones column per head (row 64 of PV psum
                                  then accumulates the softmax denominator)
  scoresT[k, q] = KT_h.T @ QT_h  (q in s16-major order so PV output lands in
                                  the layout the final reshape needs)
  wT = exp(0.125 * scoresT)      (mask is a no-op unless mask@mask.T has
                                  zeros; host checks and enables a penalty-mult
                                  fallback path in that case)
  xT'[dd|sum, q] = V_aug.T @ wT  (accumulated over k tiles)
  lhsT = xT'[0:64] * (1/sum)     (DVE copy into x_block.T layout, 2 heads
                                  side by side)
  out  = lhsT.T @ WoT            -> [128 rows, 1024] per head pair, DMA'd out.
"""

import contextlib

import numpy as np
import ml_dtypes

import concourse.bass as bass
import concourse.bacc as bacc
import concourse.tile as tile
from concourse import mybir
from concourse.bass_utils import run_bass_kernel_spmd

F32 = mybir.dt.float32
BF16 = mybir.dt.bfloat16
NP_BF16 = ml_dtypes.bfloat16



B, S, D, H = 4, 1024, 1024, 16
d_head = D // H  # 64
HPC = 8          # heads per core
JC = HPC * d_head  # 512 columns of W per core

_cached = {}


def build_program(use_mask: bool, loop_n=None, loads_in_loop=True,
                  phase='full'):
    nc = bacc.Bacc(None, target_bir_lowering=False, debug=False)

    xqT = nc.dram_tensor("xqT", [D, S], BF16, kind="ExternalInput").ap()
    xkT = nc.dram_tensor("xkT", [D, S], BF16, kind="ExternalInput").ap()
    xvT = nc.dram_tensor("xvT", [D, S], BF16, kind="ExternalInput").ap()
    wqT = nc.dram_tensor("wqT", [D, JC], BF16, kind="ExternalInput").ap()
    wkT = nc.dram_tensor("wkT", [D, JC], BF16, kind="ExternalInput").ap()
    wvT = nc.dram_tensor("wvT", [D, JC], BF16, kind="ExternalInput").ap()
    bq_col = nc.dram_tensor("bq_col", [128, 4], F32, kind="ExternalInput").ap()
    bk_col = nc.dram_tensor("bk_col", [128, 4], F32, kind="ExternalInput").ap()
    bv_bc = nc.dram_tensor("bv_bc", [128, JC], F32, kind="ExternalInput").ap()
    woT = nc.dram_tensor("woT", [D, D], BF16, kind="ExternalInput").ap()
    if use_mask:
        pen = nc.dram_tensor("pen", [S, S], BF16, kind="ExternalInput").ap()
    out = nc.dram_tensor("out", [JC, D], F32, kind="ExternalOutput").ap()

    with tile.TileContext(nc) as tc:
        with (
            tc.tile_pool(name="big", bufs=8) as big,     # x + wo tiles, 1MB bf16
            tc.tile_pool(name="wp", bufs=6) as wp,       # w half-tiles, 0.5MB
            tc.tile_pool(name="penp", bufs=2) as penp,
            tc.tile_pool(name="qt", bufs=4) as qt_p,
            tc.tile_pool(name="kt", bufs=4) as kt_p,
            tc.tile_pool(name="va", bufs=8) as va_p,
            tc.tile_pool(name="wT", bufs=8) as wT_p,
            tc.tile_pool(name="lh", bufs=2) as lh_p,
            tc.tile_pool(name="outp", bufs=2) as outp,
            tc.tile_pool(name="small", bufs=6) as smallp,
            tc.tile_pool(name="psA", bufs=2, space="PSUM") as psA,
            tc.tile_pool(name="psB", bufs=4, space="PSUM") as psB,
        ):
            def load_wide(dram, pool, tag, ncols, rows_per_tile, n, eng):
                # [128, rows_per_tile*ncols] tiles: row-blocks (contraction
                # tiles dt) interleaved along free so each DRAM matrix
                # needs few big DMAs
                a = rows_per_tile
                ts = []
                for i in range(n):
                    t = pool.tile([128, a * ncols], BF16, tag=tag, name=tag)
                    src_ap = (dram[i * a * 128:(i + 1) * a * 128, :]
                              .rearrange("(a p) s -> p a s", a=a))
                    eng.dma_start(t[:], src_ap)
                    ts.append(t)
                return lambda dt: ts[dt // a][:, (dt % a) * ncols:
                                              (dt % a + 1) * ncols]

            def emit_loads():
                ld = {}
                # biases lead the scalar ring (tiny, needed first)
                ld["bq"] = smallp.tile([128, 4], F32, tag="bias", bufs=2, name="bq_sb")
                nc.scalar.dma_start(ld["bq"][:], bq_col[:])
                ld["bk"] = smallp.tile([128, 4], F32, tag="bias", bufs=2, name="bk_sb")
                nc.scalar.dma_start(ld["bk"][:], bk_col[:])
                ld["bv"] = smallp.tile([128, JC], F32, tag="biasr", bufs=1, name="bv_sb")
                nc.scalar.dma_start(ld["bv"][:], bv_bc[:])
                # bulk streams split across the two HWDGE rings, in
                # consumption order per ring
                ld["wq"] = load_wide(wqT, wp, "w", JC, 4, 2, nc.scalar)
                ld["xq"] = load_wide(xqT, big, "x", S, 4, 2, nc.sync)
                ld["wk"] = load_wide(wkT, wp, "w", JC, 4, 2, nc.scalar)
                ld["xk"] = load_wide(xkT, big, "x", S, 4, 2, nc.sync)
                ld["wv"] = load_wide(wvT, wp, "w", JC, 4, 2, nc.sync)
                ld["xv"] = load_wide(xvT, big, "x", S, 4, 2, nc.scalar)
                ld["wo"] = load_wide(woT, big, "x", D, 4, 2, nc.scalar)
                if use_mask:
                    ld["pen"] = load_wide(pen, penp, "pen", S, 4, 2, nc.gpsimd)
                return ld

            if not loads_in_loop:
                LD = emit_loads()

            _loop = tc.For_i(0, loop_n) if loop_n else contextlib.nullcontext()
            with _loop:
                if loads_in_loop:
                    LD = emit_loads()
                wt_q, xt_q = LD["wq"], LD["xq"]
                wt_k, xt_k = LD["wk"], LD["xk"]
                wvt, xvt = LD["wv"], LD["xv"]
                wo_t = LD["wo"]
                bq_sb, bk_sb, bv_sb = LD["bq"], LD["bk"], LD["bv"]
                pen_t = LD.get("pen")

                warm = smallp.tile([1, 8], F32, tag="warm", bufs=1)
                nc.vector.memset(warm[:], 0.0)
                nc.scalar.activation(warm[:], warm[:],
                                     mybir.ActivationFunctionType.Exp)

                # V_aug tiles: stable slots; ones columns seeded early (the
                # per-iteration bias-add only rewrites the 64 value columns)
                VAS = 128  # head stride: cols 64-127 stay 1.0 -> 64 denominator copies
                VA = [va_p.tile([128, 8 * VAS], BF16, name="va") for _ in range(8)]
                for va in VA:
                    nc.vector.memset(va[:], 1.0)

                def proj_jt(wt, xt, bias_sb, dst, jt):
                    # one [128,1024] psA tile per jt; st halves; dt outer so
                    # each weight tile is loaded once for both st matmuls
                    ps = psA.tile([128, 1024], F32, tag="sc", name="ps")
                    for dt in range(8):
                        w_ap = wt(dt)[:, jt * 128:(jt + 1) * 128]
                        for st in range(2):
                            nc.tensor.matmul(
                                ps[:, st * 512:(st + 1) * 512],
                                lhsT=w_ap,
                                rhs=xt(dt)[:, st * 512:(st + 1) * 512],
                                start=(dt == 0),
                                stop=(dt == 7),
                            )
                    for st in range(2):
                        nc.vector.tensor_scalar_add(
                            dst[jt][:, st * 512:(st + 1) * 512],
                            ps[:, st * 512:(st + 1) * 512],
                            bias_sb[:, jt:jt + 1],
                        )

                def proj_qk(wt, xt, bias_sb, dst_pool):
                    dst = [dst_pool.tile([128, S], BF16, tag="dst", name="dst") for _ in range(4)]
                    for jt in range(4):
                        proj_jt(wt, xt, bias_sb, dst, jt)
                    return dst

                QT = proj_qk(wt_q, xt_q, bq_sb, qt_p)
                KT = proj_qk(wt_k, xt_k, bk_sb, kt_p)

                # ---- V projection -> V_aug [s, 8*65] (65th col per head = 1.0)
                for sp in range(4):  # st pairs; one psA tile per pair
                    ps = psA.tile([128, 1024], F32, tag="sc", name="vps")
                    for half in range(2):
                        st = 2 * sp + half
                        for dt in range(8):
                            nc.tensor.matmul(
                                ps[:, half * 512:(half + 1) * 512],
                                lhsT=xvt(dt)[:, st * 128:(st + 1) * 128],
                                rhs=wvt(dt),
                                start=(dt == 0),
                                stop=(dt == 7),
                            )
                    for half in range(2):
                        st = 2 * sp + half
                        nc.vector.tensor_tensor(
                            VA[st][:].rearrange("p (h c) -> p h c", h=8)[:, :, 0:64],
                            ps[:, half * 512:(half + 1) * 512]
                                .rearrange("p (h c) -> p h c", h=8),
                            bv_sb[:].rearrange("p (h c) -> p h c", h=8),
                            op=mybir.AluOpType.add,
                        )


                def QT_perm(hl, qch):
                    # rhs [64, 512] with q in s16-major order:
                    # col j reads s = q16*16 + s16, s16 = qch*8 + j//64, q16 = j%64
                    tile_ = QT[hl // 2]
                    po = (hl % 2) * 64
                    ap = tile_[po:po + 64, :].rearrange("p (q s) -> p s q", s=16)
                    return ap[:, qch * 8:(qch + 1) * 8, :]

                def KT_ap(hl, kt):
                    tile_ = KT[hl // 2]
                    po = (hl % 2) * 64
                    return tile_[po:po + 64, kt * 128:(kt + 1) * 128]

                PV_LAG = 3  # kt-steps the PV matmuls trail scores/exp

                def attention(p, hook_norm=None, hook_fp=None, lag=None):
                    lag = PV_LAG if lag is None else lag
                    hA, hB = 2 * p, 2 * p + 1
                    pv = {}
                    wstash = {}
                    for step in range(8 + lag):
                        if step == 1 and hook_norm is not None:
                            hook_norm()
                        if step == 4 and hook_fp is not None:
                            hook_fp()
                        if step < 8:
                            kt = step
                            scA = psA.tile([128, 1024], F32, tag="sc")
                            scB = psA.tile([128, 1024], F32, tag="sc")
                            # per head: both qch back-to-back (shared lhsT,
                            # and exp can start after the 2nd matmul)
                            for hl, sc in ((hA, scA), (hB, scB)):
                                for qch in range(2):
                                    nc.tensor.matmul(
                                        sc[:, qch * 512:(qch + 1) * 512],
                                        lhsT=KT_ap(hl, kt),
                                        rhs=QT_perm(hl, qch),
                                        start=True, stop=True,
                                    )
                            wA = wT_p.tile([128, 1024], BF16, tag="wT")
                            wB = wT_p.tile([128, 1024], BF16, tag="wT")
                            nc.scalar.activation(wA[:], scA[:],
                                                 mybir.ActivationFunctionType.Exp,
                                                 scale=0.125)
                            nc.scalar.activation(wB[:], scB[:],
                                                 mybir.ActivationFunctionType.Exp,
                                                 scale=0.125)
                            if use_mask:
                                # multiply by the 0/1 keep-mask (pen[k, q]) with
                                # the same s16-major q permutation as wT columns
                                pap = pen_t(kt).rearrange("p (q s) -> p s q", s=16)
                                for w_ in (wA, wB):
                                    nc.vector.tensor_tensor(
                                        w_[:].rearrange("p (s q) -> p s q", s=16),
                                        w_[:].rearrange("p (s q) -> p s q", s=16),
                                        pap, op=mybir.AluOpType.mult,
                                    )
                            wstash[kt] = (wA, wB)
                        if phase in ('proj', 'scexp'):
                            wstash.clear()
                            continue
                        if step >= lag:
                            kt = step - lag
                            wA, wB = wstash.pop(kt)
                            # per head: both qch consecutive (shared VA lhsT);
                            # pv[i] key: i = 2*hloc + qch
                            for i, (hl, wt_, qch) in enumerate(
                                [(hA, wA, 0), (hA, wA, 1), (hB, wB, 0), (hB, wB, 1)]
                            ):
                                if kt == 0:
                                    pv[i] = psB.tile([128, 512], F32, tag="ps1", name="pv")
                                nc.tensor.matmul(
                                    pv[i][:],
                                    lhsT=VA[kt][:, hl * VAS:(hl + 1) * VAS],
                                    rhs=wt_[:, qch * 512:(qch + 1) * 512],
                                    start=(kt == 0), stop=(kt == 7),
                                )
                    return pv

                def tail_norm(p, pv):
                    hA, hB = 2 * p, 2 * p + 1
                    # pv[i]: i = 2*hloc + qch
                    # normalize + shuffle into final-projection lhsT layout
                    lh = lh_p.tile([128, 1024], BF16)
                    for hloc, hl in enumerate((hA, hB)):
                        rs = smallp.tile([64, 1024], F32, tag="rs", bufs=2,
                                         name="rs")
                        for qch in range(2):
                            i = 2 * hloc + qch
                            nc.vector.reciprocal(
                                rs[:, qch * 512:(qch + 1) * 512],
                                pv[i][64:128, :])
                        rcv = rs[:].rearrange("p (s q) -> p s q", s=16)
                        for qch in range(2):
                            i = 2 * hloc + qch
                            src = pv[i][0:64, :].rearrange("p (s q) -> p s q", s=8)
                            for par, off in ((0, 0), (1, 64)):  # even/odd s16
                                # lh layout: [part, (ct 8)(head 2)(q16 64)] so the
                                # final matmul's lhsT tile ct is one contiguous
                                # 128-col block (walrus: stationary AP needs a
                                # single free dim)
                                dst = lh[off:off + 64, :].rearrange(
                                    "p (c m) -> p c m", c=8
                                )[:, qch * 4:(qch + 1) * 4,
                                  hloc * 64:(hloc + 1) * 64]
                                nc.vector.tensor_tensor(
                                    dst,
                                    src[:, par::2, :],
                                    rcv[:, qch * 8 + par:qch * 8 + 8:2, :],
                                    op=mybir.AluOpType.mult,
                                )

                    return lh

                def tail_fp(p, lh):
                    # final projection: out rows p*128 .. p*128+128
                    # one [128,1024] psA tile; ot halves; ct outer so each
                    # lh slice is loaded once for both ot matmuls
                    ob = outp.tile([128, 1024], F32)
                    fp = psA.tile([128, 1024], F32, tag="sc", name="fp")
                    for ct in range(8):
                        lh_ap = lh[:, ct * 128:(ct + 1) * 128]
                        for ot in range(2):
                            nc.tensor.matmul(
                                fp[:, ot * 512:(ot + 1) * 512],
                                lhsT=lh_ap,
                                rhs=wo_t(ct)[:, ot * 512:(ot + 1) * 512],
                                start=(ct == 0), stop=(ct == 7),
                            )
                    for ot in range(2):
                        nc.vector.tensor_copy(
                            ob[:, ot * 512:(ot + 1) * 512],
                            fp[:, ot * 512:(ot + 1) * 512])
                    nc.sync.dma_start(out[p * 128:(p + 1) * 128, :], ob[:])

                if phase == 'proj':
                    dbg = outp.tile([128, 1024], F32, name="dbg")
                    nc.vector.tensor_copy(dbg[:, 0:260], VA[0][:].bitcast(F32)[:, 0:260])
                    nc.vector.tensor_copy(dbg[:, 0:512], QT[0][:].bitcast(F32)[:, 0:512])
                    nc.vector.tensor_copy(dbg[:, 0:512], KT[0][:].bitcast(F32)[:, 0:512])
                    nc.sync.dma_start(out[0:128, :], dbg[:])
                elif phase == 'scexp':
                    for p in range(4):
                        attention(p)
                    dbg = outp.tile([128, 1024], F32, name="dbg")
                    nc.vector.tensor_copy(dbg[:, 0:260], VA[0][:].bitcast(F32)[:, 0:260])
                    nc.sync.dma_start(out[0:128, :], dbg[:])
                elif phase == 'pvonly':
                    # PV included; drain pv accumulators with cheap DVE copies
                    dbg = outp.tile([128, 1024], F32, name="dbg")
                    for p in range(4):
                        pv = attention(p)
                        for i in range(4):
                            nc.vector.tensor_copy(dbg[0:65, 0:512], pv[i][:])
                    nc.sync.dma_start(out[0:128, :], dbg[:])
                else:
                    # software-pipeline: emit pair p's attention before pair
                    # p-1's norm/final so PE gap-fills the ACT-paced exp phase
                    pending = None
                    for p in range(4):
                        hn = hf = None
                        if pending is not None:
                            pp, ppv = pending
                            box = {}

                            def hn(pp=pp, ppv=ppv, box=box):
                                box["lh"] = tail_norm(pp, ppv)

                            def hf(pp=pp, box=box):
                                tail_fp(pp, box["lh"])

                        pv = attention(p, hn, hf)
                        pending = (p, pv)
                    pp, ppv = pending
                    tail_fp(pp, tail_norm(pp, ppv))

    nc.compile()
    return nc


def make_in_maps(query, key, value, mask, Wq, bq, Wk, bk, Wv, bv, Wo,
                 pen_b=None):
    woT = np.ascontiguousarray(Wo.T).astype(NP_BF16)
    maps = []
    for c in range(8):
        b, hf = c // 2, c % 2
        sl = slice(hf * JC, (hf + 1) * JC)
        m = {
            "xqT": np.ascontiguousarray(query[b].T).astype(NP_BF16),
            "xkT": np.ascontiguousarray(key[b].T).astype(NP_BF16),
            "xvT": np.ascontiguousarray(value[b].T).astype(NP_BF16),
            "wqT": np.ascontiguousarray(Wq[sl].T).astype(NP_BF16),
            "wkT": np.ascontiguousarray(Wk[sl].T).astype(NP_BF16),
            "wvT": np.ascontiguousarray(Wv[sl].T).astype(NP_BF16),
            "bq_col": np.ascontiguousarray(bq[sl].reshape(4, 128).T),
            "bk_col": np.ascontiguousarray(bk[sl].reshape(4, 128).T),
            "bv_bc": np.ascontiguousarray(
                np.broadcast_to(bv[sl].reshape(1, JC), (128, JC))),
            "woT": woT,
        }
        if pen_b is not None:
            m["pen"] = pen_b[b].astype(NP_BF16)
        maps.append(m)
    return maps


def kernel(query, key, value, mask, Wq, bq, Wk, bk, Wv, bv, Wo):
    query = np.asarray(query, np.float32)
    key = np.asarray(key, np.float32)
    value = np.asarray(value, np.float32)
    mask = np.asarray(mask, np.float32)

    m2d = mask[0]  # [B, S, 64]
    mm = np.stack([m2d[b] @ m2d[b].T for b in range(B)])  # [B, S, S]
    use_mask = bool((mm == 0).any())
    pen_b = None
    if use_mask:
        pen_b = np.where(mm == 0, np.float32(0.0), np.float32(1.0))
        pen_b = np.ascontiguousarray(pen_b, np.float32)

    if use_mask not in _cached:
        _cached[use_mask] = build_program(use_mask)
    nc = _cached[use_mask]

    in_maps = make_in_maps(query, key, value, mask,
                           np.asarray(Wq, np.float32), np.asarray(bq, np.float32),
                           np.asarray(Wk, np.float32), np.asarray(bk, np.float32),
                           np.asarray(Wv, np.float32), np.asarray(bv, np.float32),
                           np.asarray(Wo, np.float32), pen_b)
    res = run_bass_kernel_spmd(nc, in_maps, list(range(8)))

    out = np.empty((B, S, D), np.float32)
    for c in range(8):
        b, hf = c // 2, c % 2
        out[b, hf * JC:(hf + 1) * JC, :] = res.results[c]["out"]
    return out


# revision 21
# speedup vs baseline: 1.3403x; 1.2640x over previous
"""Trainium2 Bass kernel for nn_MultiHeadedAttention (B=4, S=1024, D=1024, H=16).

Sharding: 8 cores = 4 batches x 2 head-halves (8 heads each). The reference's
row-major reshape after [B,H,S,d] means output row r = h*64 + s//16 depends
only on head h, so head sharding needs no collective: each core computes a
[512, 1024] row-block of its batch's output.

All matmul operands are bf16 (host-converted): same PE rate as fp32r
(1 cycle/row at N>=256) but half the HBM traffic (~12MB/core), and no
in-flight dtype cast -- so every bulk load rides the two hardware DGE
rings (SP + Activation) instead of the slow gpsimd SWDGE ring. Pools are
sized so all input DMAs are in flight from t=0 with no slot recycling.

PSUM layout: psA (2 x [128,1024]) hosts the projection chunks, the score
tiles, and the final-projection accumulators; psB (4 x [65,512]) hosts
only the four PV accumulators of the current pair, so the pv ring never
couples to proj/fp allocations. Matmuls are ordered so consecutive
instructions share the stationary operand (one ldweights per 2 matmuls
in proj/scores/final).

Per-core pipeline (all matmuls contract on the partition dim):
  QT/KT = WxT.T @ XxT          -> [j, s] layout (head dims on partitions)
  V     = XvT.T @ WvT          -> [s, j] natural layout, augmented with a
                                  ones column per head (row 64 of PV psum
                                  then accumulates the softmax denominator)
  scoresT[k, q] = KT_h.T @ QT_h  (q in s16-major order so PV output lands in
                                  the layout the final reshape needs)
  wT = exp(0.125 * scoresT)      (mask is a no-op unless mask@mask.T has
                                  zeros; host checks and enables a penalty-mult
                                  fallback path in that case)
  xT'[dd|sum, q] = V_aug.T @ wT  (accumulated over k tiles)
  lhsT = xT'[0:64] * (1/sum)     (DVE copy into x_block.T layout, 2 heads
                                  side by side)
  out  = lhsT.T @ WoT            -> [128 rows, 1024] per head pair, DMA'd out.
"""

import contextlib

import numpy as np
import ml_dtypes

import concourse.bass as bass
import concourse.bacc as bacc
import concourse.tile as tile
from concourse import mybir
from concourse.bass_utils import run_bass_kernel_spmd

F32 = mybir.dt.float32
BF16 = mybir.dt.bfloat16
NP_BF16 = ml_dtypes.bfloat16



B, S, D, H = 4, 1024, 1024, 16
d_head = D // H  # 64
HPC = 8          # heads per core
JC = HPC * d_head  # 512 columns of W per core

_cached = {}


def build_program(use_mask: bool, loop_n=None, loads_in_loop=True,
                  phase='full'):
    nc = bacc.Bacc(None, target_bir_lowering=False, debug=False)

    xqT = nc.dram_tensor("xqT", [D, S], BF16, kind="ExternalInput").ap()
    xkT = nc.dram_tensor("xkT", [D, S], BF16, kind="ExternalInput").ap()
    xvT = nc.dram_tensor("xvT", [D, S], BF16, kind="ExternalInput").ap()
    wqT = nc.dram_tensor("wqT", [D, JC], BF16, kind="ExternalInput").ap()
    wkT = nc.dram_tensor("wkT", [D, JC], BF16, kind="ExternalInput").ap()
    wvT = nc.dram_tensor("wvT", [D, JC], BF16, kind="ExternalInput").ap()
    bq_col = nc.dram_tensor("bq_col", [128, 4], F32, kind="ExternalInput").ap()
    bk_col = nc.dram_tensor("bk_col", [128, 4], F32, kind="ExternalInput").ap()
    bv_bc = nc.dram_tensor("bv_bc", [128, JC], F32, kind="ExternalInput").ap()
    woT = nc.dram_tensor("woT", [D, D], BF16, kind="ExternalInput").ap()
    if use_mask:
        pen = nc.dram_tensor("pen", [S, S], BF16, kind="ExternalInput").ap()
    out = nc.dram_tensor("out", [JC, D], F32, kind="ExternalOutput").ap()

    with tile.TileContext(nc) as tc:
        with (
            tc.tile_pool(name="big", bufs=8) as big,     # x + wo tiles, 1MB bf16
            tc.tile_pool(name="wp", bufs=6) as wp,       # w half-tiles, 0.5MB
            tc.tile_pool(name="penp", bufs=2) as penp,
            tc.tile_pool(name="qt", bufs=4) as qt_p,
            tc.tile_pool(name="kt", bufs=4) as kt_p,
            tc.tile_pool(name="va", bufs=8) as va_p,
            tc.tile_pool(name="wT", bufs=6) as wT_p,
            tc.tile_pool(name="lh", bufs=2) as lh_p,
            tc.tile_pool(name="outp", bufs=2) as outp,
            tc.tile_pool(name="small", bufs=6) as smallp,
            tc.tile_pool(name="psA", bufs=2, space="PSUM") as psA,
            tc.tile_pool(name="psB", bufs=4, space="PSUM") as psB,
        ):
            def load_wide(dram, pool, tag, ncols, rows_per_tile, n, eng):
                # [128, rows_per_tile*ncols] tiles: row-blocks (contraction
                # tiles dt) interleaved along free so each DRAM matrix
                # needs few big DMAs
                a = rows_per_tile
                ts = []
                for i in range(n):
                    t = pool.tile([128, a * ncols], BF16, tag=tag, name=tag)
                    src_ap = (dram[i * a * 128:(i + 1) * a * 128, :]
                              .rearrange("(a p) s -> p a s", a=a))
                    eng.dma_start(t[:], src_ap)
                    ts.append(t)
                return lambda dt: ts[dt // a][:, (dt % a) * ncols:
                                              (dt % a + 1) * ncols]

            def emit_loads():
                ld = {}
                # biases lead the scalar ring (tiny, needed first)
                ld["bq"] = smallp.tile([128, 4], F32, tag="bias", bufs=2, name="bq_sb")
                nc.scalar.dma_start(ld["bq"][:], bq_col[:])
                ld["bk"] = smallp.tile([128, 4], F32, tag="bias", bufs=2, name="bk_sb")
                nc.scalar.dma_start(ld["bk"][:], bk_col[:])
                ld["bv"] = smallp.tile([128, JC], F32, tag="biasr", bufs=1, name="bv_sb")
                nc.scalar.dma_start(ld["bv"][:], bv_bc[:])
                # bulk streams split across the two HWDGE rings, in
                # consumption order per ring
                ld["wq"] = load_wide(wqT, wp, "w", JC, 4, 2, nc.scalar)
                ld["xq"] = load_wide(xqT, big, "x", S, 4, 2, nc.sync)
                ld["wk"] = load_wide(wkT, wp, "w", JC, 4, 2, nc.scalar)
                ld["xk"] = load_wide(xkT, big, "x", S, 4, 2, nc.sync)
                ld["wv"] = load_wide(wvT, wp, "w", JC, 4, 2, nc.sync)
                ld["xv"] = load_wide(xvT, big, "x", S, 4, 2, nc.scalar)
                ld["wo"] = load_wide(woT, big, "x", D, 4, 2, nc.scalar)
                if use_mask:
                    ld["pen"] = load_wide(pen, penp, "pen", S, 4, 2, nc.gpsimd)
                return ld

            if not loads_in_loop:
                LD = emit_loads()

            _loop = tc.For_i(0, loop_n) if loop_n else contextlib.nullcontext()
            with _loop:
                if loads_in_loop:
                    LD = emit_loads()
                wt_q, xt_q = LD["wq"], LD["xq"]
                wt_k, xt_k = LD["wk"], LD["xk"]
                wvt, xvt = LD["wv"], LD["xv"]
                wo_t = LD["wo"]
                bq_sb, bk_sb, bv_sb = LD["bq"], LD["bk"], LD["bv"]
                pen_t = LD.get("pen")

                warm = smallp.tile([1, 8], F32, tag="warm", bufs=1)
                nc.vector.memset(warm[:], 0.0)
                nc.scalar.activation(warm[:], warm[:],
                                     mybir.ActivationFunctionType.Exp)

                # V_aug tiles: stable slots; ones columns seeded early (the
                # per-iteration bias-add only rewrites the 64 value columns)
                VAS = 128  # head stride: cols 64-127 stay 1.0 -> 64 denominator copies
                VA = [va_p.tile([128, 8 * VAS], BF16, name="va") for _ in range(8)]
                for va in VA:
                    nc.vector.memset(va[:], 1.0)

                def proj_jt(wt, xt, bias_sb, dst, jt):
                    # one [128,1024] psA tile per jt; st halves; dt outer so
                    # each weight tile is loaded once for both st matmuls
                    ps = psA.tile([128, 1024], F32, tag="sc", name="ps")
                    for dt in range(8):
                        w_ap = wt(dt)[:, jt * 128:(jt + 1) * 128]
                        for st in range(2):
                            nc.tensor.matmul(
                                ps[:, st * 512:(st + 1) * 512],
                                lhsT=w_ap,
                                rhs=xt(dt)[:, st * 512:(st + 1) * 512],
                                start=(dt == 0),
                                stop=(dt == 7),
                            )
                    for st in range(2):
                        nc.vector.tensor_scalar_add(
                            dst[jt][:, st * 512:(st + 1) * 512],
                            ps[:, st * 512:(st + 1) * 512],
                            bias_sb[:, jt:jt + 1],
                        )

                def proj_qk(wt, xt, bias_sb, dst_pool):
                    dst = [dst_pool.tile([128, S], BF16, tag="dst", name="dst") for _ in range(4)]
                    for jt in range(4):
                        proj_jt(wt, xt, bias_sb, dst, jt)
                    return dst

                QT = proj_qk(wt_q, xt_q, bq_sb, qt_p)
                KT = proj_qk(wt_k, xt_k, bk_sb, kt_p)

                # ---- V projection -> V_aug [s, 8*65] (65th col per head = 1.0)
                for sp in range(4):  # st pairs; one psA tile per pair
                    ps = psA.tile([128, 1024], F32, tag="sc", name="vps")
                    for half in range(2):
                        st = 2 * sp + half
                        for dt in range(8):
                            nc.tensor.matmul(
                                ps[:, half * 512:(half + 1) * 512],
                                lhsT=xvt(dt)[:, st * 128:(st + 1) * 128],
                                rhs=wvt(dt),
                                start=(dt == 0),
                                stop=(dt == 7),
                            )
                    for half in range(2):
                        st = 2 * sp + half
                        nc.vector.tensor_tensor(
                            VA[st][:].rearrange("p (h c) -> p h c", h=8)[:, :, 0:64],
                            ps[:, half * 512:(half + 1) * 512]
                                .rearrange("p (h c) -> p h c", h=8),
                            bv_sb[:].rearrange("p (h c) -> p h c", h=8),
                            op=mybir.AluOpType.add,
                        )


                def QT_perm(hl, qch):
                    # xqT columns are host-permuted to s16-major order
                    # (g = s16*64 + q16), so the rhs is a contiguous slice
                    tile_ = QT[hl // 2]
                    po = (hl % 2) * 64
                    return tile_[po:po + 64, qch * 512:(qch + 1) * 512]

                def KT_ap(hl, kt):
                    tile_ = KT[hl // 2]
                    po = (hl % 2) * 64
                    return tile_[po:po + 64, kt * 128:(kt + 1) * 128]

                PV_LAG = 2  # kt-steps the PV matmuls trail scores/exp

                def attention(p, hook_norm=None, hook_fp=None, lag=None):
                    lag = PV_LAG if lag is None else lag
                    hA, hB = 2 * p, 2 * p + 1
                    pv = {}
                    wstash = {}
                    for step in range(8 + lag):
                        if step == 1 and hook_norm is not None:
                            hook_norm()
                        if step == 4 and hook_fp is not None:
                            hook_fp()
                        if step < 8:
                            kt = step
                            scA = psA.tile([128, 1024], F32, tag="sc")
                            scB = psA.tile([128, 1024], F32, tag="sc")
                            # per head: both qch back-to-back (shared lhsT,
                            # and exp can start after the 2nd matmul)
                            for hl, sc in ((hA, scA), (hB, scB)):
                                for qch in range(2):
                                    nc.tensor.matmul(
                                        sc[:, qch * 512:(qch + 1) * 512],
                                        lhsT=KT_ap(hl, kt),
                                        rhs=QT_perm(hl, qch),
                                        start=True, stop=True,
                                    )
                            wA = wT_p.tile([128, 1024], BF16, tag="wT")
                            wB = wT_p.tile([128, 1024], BF16, tag="wT")
                            nc.scalar.activation(wA[:], scA[:],
                                                 mybir.ActivationFunctionType.Exp,
                                                 scale=0.125)
                            nc.scalar.activation(wB[:], scB[:],
                                                 mybir.ActivationFunctionType.Exp,
                                                 scale=0.125)
                            if use_mask:
                                # pen columns host-permuted like wT columns
                                for w_ in (wA, wB):
                                    nc.vector.tensor_tensor(
                                        w_[:], w_[:], pen_t(kt),
                                        op=mybir.AluOpType.mult,
                                    )
                            wstash[kt] = (wA, wB)
                        if phase in ('proj', 'scexp'):
                            wstash.clear()
                            continue
                        if step >= lag:
                            kt = step - lag
                            wA, wB = wstash.pop(kt)
                            # per head: both qch consecutive (shared VA lhsT);
                            # pv[i] key: i = 2*hloc + qch
                            for i, (hl, wt_, qch) in enumerate(
                                [(hA, wA, 0), (hA, wA, 1), (hB, wB, 0), (hB, wB, 1)]
                            ):
                                if kt == 0:
                                    pv[i] = psB.tile([128, 512], F32, tag="ps1", name="pv")
                                nc.tensor.matmul(
                                    pv[i][:],
                                    lhsT=VA[kt][:, hl * VAS:(hl + 1) * VAS],
                                    rhs=wt_[:, qch * 512:(qch + 1) * 512],
                                    start=(kt == 0), stop=(kt == 7),
                                )
                    return pv

                def tail_norm(p, pv):
                    hA, hB = 2 * p, 2 * p + 1
                    # pv[i]: i = 2*hloc + qch
                    # normalize + shuffle into final-projection lhsT layout
                    lh = lh_p.tile([128, 1024], BF16)
                    for hloc, hl in enumerate((hA, hB)):
                        rs = smallp.tile([64, 1024], F32, tag="rs", bufs=2,
                                         name="rs")
                        for qch in range(2):
                            i = 2 * hloc + qch
                            nc.vector.reciprocal(
                                rs[:, qch * 512:(qch + 1) * 512],
                                pv[i][64:128, :])
                        rcv = rs[:].rearrange("p (s q) -> p s q", s=16)
                        for qch in range(2):
                            i = 2 * hloc + qch
                            src = pv[i][0:64, :].rearrange("p (s q) -> p s q", s=8)
                            for par, off in ((0, 0), (1, 64)):  # even/odd s16
                                # lh layout: [part, (ct 8)(head 2)(q16 64)] so the
                                # final matmul's lhsT tile ct is one contiguous
                                # 128-col block (walrus: stationary AP needs a
                                # single free dim)
                                dst = lh[off:off + 64, :].rearrange(
                                    "p (c m) -> p c m", c=8
                                )[:, qch * 4:(qch + 1) * 4,
                                  hloc * 64:(hloc + 1) * 64]
                                nc.vector.tensor_tensor(
                                    dst,
                                    src[:, par::2, :],
                                    rcv[:, qch * 8 + par:qch * 8 + 8:2, :],
                                    op=mybir.AluOpType.mult,
                                )

                    return lh

                def tail_fp(p, lh):
                    # final projection: out rows p*128 .. p*128+128
                    # one [128,1024] psA tile; ot halves; ct outer so each
                    # lh slice is loaded once for both ot matmuls
                    ob = outp.tile([128, 1024], F32)
                    fp = psA.tile([128, 1024], F32, tag="sc", name="fp")
                    for ct in range(8):
                        lh_ap = lh[:, ct * 128:(ct + 1) * 128]
                        for ot in range(2):
                            nc.tensor.matmul(
                                fp[:, ot * 512:(ot + 1) * 512],
                                lhsT=lh_ap,
                                rhs=wo_t(ct)[:, ot * 512:(ot + 1) * 512],
                                start=(ct == 0), stop=(ct == 7),
                            )
                    for ot in range(2):
                        nc.vector.tensor_copy(
                            ob[:, ot * 512:(ot + 1) * 512],
                            fp[:, ot * 512:(ot + 1) * 512])
                    nc.sync.dma_start(out[p * 128:(p + 1) * 128, :], ob[:])

                if phase == 'proj':
                    dbg = outp.tile([128, 1024], F32, name="dbg")
                    nc.vector.tensor_copy(dbg[:, 0:260], VA[0][:].bitcast(F32)[:, 0:260])
                    nc.vector.tensor_copy(dbg[:, 0:512], QT[0][:].bitcast(F32)[:, 0:512])
                    nc.vector.tensor_copy(dbg[:, 0:512], KT[0][:].bitcast(F32)[:, 0:512])
                    nc.sync.dma_start(out[0:128, :], dbg[:])
                elif phase == 'scexp':
                    for p in range(4):
                        attention(p)
                    dbg = outp.tile([128, 1024], F32, name="dbg")
                    nc.vector.tensor_copy(dbg[:, 0:260], VA[0][:].bitcast(F32)[:, 0:260])
                    nc.sync.dma_start(out[0:128, :], dbg[:])
                elif phase == 'pvonly':
                    # PV included; drain pv accumulators with cheap DVE copies
                    dbg = outp.tile([128, 1024], F32, name="dbg")
                    for p in range(4):
                        pv = attention(p)
                        for i in range(4):
                            nc.vector.tensor_copy(dbg[0:65, 0:512], pv[i][:])
                    nc.sync.dma_start(out[0:128, :], dbg[:])
                else:
                    # software-pipeline: emit pair p's attention before pair
                    # p-1's norm/final so PE gap-fills the ACT-paced exp phase
                    pending = None
                    for p in range(4):
                        hn = hf = None
                        if pending is not None:
                            pp, ppv = pending
                            box = {}

                            def hn(pp=pp, ppv=ppv, box=box):
                                box["lh"] = tail_norm(pp, ppv)

                            def hf(pp=pp, box=box):
                                tail_fp(pp, box["lh"])

                        pv = attention(p, hn, hf)
                        pending = (p, pv)
                    pp, ppv = pending
                    tail_fp(pp, tail_norm(pp, ppv))

    nc.compile()
    return nc


QPERM = (np.arange(S) % 64) * 16 + np.arange(S) // 64  # g -> s


def make_in_maps(query, key, value, mask, Wq, bq, Wk, bk, Wv, bv, Wo,
                 pen_b=None):
    woT = np.ascontiguousarray(Wo.T).astype(NP_BF16)
    maps = []
    for c in range(8):
        b, hf = c // 2, c % 2
        sl = slice(hf * JC, (hf + 1) * JC)
        m = {
            "xqT": np.ascontiguousarray(query[b].T[:, QPERM]).astype(NP_BF16),
            "xkT": np.ascontiguousarray(key[b].T).astype(NP_BF16),
            "xvT": np.ascontiguousarray(value[b].T).astype(NP_BF16),
            "wqT": np.ascontiguousarray(Wq[sl].T).astype(NP_BF16),
            "wkT": np.ascontiguousarray(Wk[sl].T).astype(NP_BF16),
            "wvT": np.ascontiguousarray(Wv[sl].T).astype(NP_BF16),
            "bq_col": np.ascontiguousarray(bq[sl].reshape(4, 128).T),
            "bk_col": np.ascontiguousarray(bk[sl].reshape(4, 128).T),
            "bv_bc": np.ascontiguousarray(
                np.broadcast_to(bv[sl].reshape(1, JC), (128, JC))),
            "woT": woT,
        }
        if pen_b is not None:
            m["pen"] = np.ascontiguousarray(pen_b[b][:, QPERM]).astype(NP_BF16)
        maps.append(m)
    return maps


def kernel(query, key, value, mask, Wq, bq, Wk, bk, Wv, bv, Wo):
    query = np.asarray(query, np.float32)
    key = np.asarray(key, np.float32)
    value = np.asarray(value, np.float32)
    mask = np.asarray(mask, np.float32)

    m2d = mask[0]  # [B, S, 64]
    mm = np.stack([m2d[b] @ m2d[b].T for b in range(B)])  # [B, S, S]
    use_mask = bool((mm == 0).any())
    pen_b = None
    if use_mask:
        pen_b = np.where(mm == 0, np.float32(0.0), np.float32(1.0))
        pen_b = np.ascontiguousarray(pen_b, np.float32)

    if use_mask not in _cached:
        _cached[use_mask] = build_program(use_mask)
    nc = _cached[use_mask]

    in_maps = make_in_maps(query, key, value, mask,
                           np.asarray(Wq, np.float32), np.asarray(bq, np.float32),
                           np.asarray(Wk, np.float32), np.asarray(bk, np.float32),
                           np.asarray(Wv, np.float32), np.asarray(bv, np.float32),
                           np.asarray(Wo, np.float32), pen_b)
    res = run_bass_kernel_spmd(nc, in_maps, list(range(8)))

    out = np.empty((B, S, D), np.float32)
    for c in range(8):
        b, hf = c // 2, c % 2
        out[b, hf * JC:(hf + 1) * JC, :] = res.results[c]["out"]
    return out
